# revision 22
# baseline (speedup 1.0000x reference)
"""Trainium2 Bass kernel for ContextAttentionMaskLuong.

Reference computation (per batch b):
    keys  = x @ W                       [B,S,D]
    query = tanh(c @ Wc + b)            [B,D]
    eij   = scale * <query, keys_s>     [B,S]
    a     = exp(eij - max) * mask; a /= (sum(a) + 1e-7)
    out   = sum_s a[s] * x[s,:]         [B,D]

Key rewrite: eij[b,s] = <x[b,s,:], q2[b]> with q2[b] = W @ (scale*query[b]),
which removes the [B,S,D]x[D,D] matmul entirely. The kernel is then one
streaming pass over x (memory-bound, ~25MB HBM read per core).

Sharding: data-parallel over batch: 16 batches / 8 cores = 2 per core.
W/Wc/b/scale replicated.

v2 design (engine balance; DMA is the roofline at ~80us):
- query = tanh(c @ Wc + bias): PE matmuls with tiny cT as the stationary
  operand and Wc streaming in natural layout (16 matmuls, f32r 1cyc/row).
- q2 = W @ (scale*query): fused mult+reduce on DVE/GpSimd with W in
  natural [d-part, e-free] layout (no PE transposes of W at all).
- partition broadcasts via selector-matrix matmuls (lhsT row-selector).
- eij via DVE+GpSimd scalar_tensor_tensor, split between both engines.
- softmax via reduce + PE transpose trick (tiny).
- pooling out = a^T @ x via PE matmuls in f32r (1 cyc/row).
- x tile DMAs issued after W/Wc so q2 is ready when x tiles land.

Per-core x layout (s-major): x tile t of batch b is SBUF [128, 4096]
where partition p, free q*1024+d  <->  x[b, 512*t + 4*p + q, d].
"""

import numpy as np

B, S, D = 16, 2048, 1024
NCORES = 8
BPC = B // NCORES  # batches per core
EPS = 1e-7

TS = 4  # x tiles per batch
QT = 4  # s-rows per partition per tile
XF = QT * D  # x tile free size (4096)
SBLK = S // TS  # s-block per tile (512)

_CACHE = {}


def _build():
    import os

    phase = int(os.environ.get("KPHASE", "5"))
    repeat = int(os.environ.get("KREPEAT", "1"))
    import concourse.bass as bass
    import concourse.mybir as mybir
    import concourse.tile as tile
    from concourse import bacc
    from concourse.masks import make_identity

    fp32 = mybir.dt.float32
    f32r = mybir.dt.float32r
    bf16 = mybir.dt.bfloat16
    i32 = mybir.dt.int32
    AF = mybir.ActivationFunctionType
    OP = mybir.AluOpType
    ts = bass.ts

    nc = bacc.Bacc(None)

    x_d = nc.dram_tensor("x", [BPC, S, D], f32r, kind="ExternalInput")
    mask_d = nc.dram_tensor("mask", [BPC, S], i32, kind="ExternalInput")
    c_d = nc.dram_tensor("c", [BPC, D], fp32, kind="ExternalInput")
    w_d = nc.dram_tensor("W", [D, D], fp32, kind="ExternalInput")
    wc_d = nc.dram_tensor("Wc", [D, D], fp32, kind="ExternalInput")
    b_d = nc.dram_tensor("b", [D], fp32, kind="ExternalInput")
    scale_d = nc.dram_tensor("scale", [1], fp32, kind="ExternalInput")
    out_d = nc.dram_tensor("out", [BPC, D], fp32, kind="ExternalOutput")

    KD = D // 128  # 8 chunks of 128 along d/e/c

    def r(ap):  # bitcast to f32r for 1-cycle/row PE matmuls
        return ap.bitcast(f32r)

    def f(ap):  # view f32r-declared data as plain f32 for DVE/ACT ops
        return ap.bitcast(fp32)

    with tile.TileContext(nc) as tc:
        with (
            tc.tile_pool(name="const", bufs=1) as const,
            tc.tile_pool(name="xp", bufs=2 * TS) as xp,
            tc.tile_pool(name="wstream", bufs=2) as wstream,
            tc.tile_pool(name="persist", bufs=1) as persist,
            tc.tile_pool(name="scratch", bufs=3) as scratch,
            tc.tile_pool(name="pq", bufs=1, space="PSUM") as pq,
            tc.tile_pool(name="pb", bufs=2, space="PSUM") as pb,
            tc.tile_pool(name="pp", bufs=2, space="PSUM") as pp,
            tc.tile_pool(name="pstt", bufs=1, space="PSUM") as pstt,
        ):
            # ---------- constants / small loads ----------
            identity = const.tile([128, 128], fp32, tag="identity")
            make_identity(nc, identity)
            ones1 = const.tile([1, 128], fp32, tag="ones1")
            nc.vector.memset(ones1, 1.0)
            ones_col = const.tile([128, 1], fp32, tag="ones_col")
            nc.vector.memset(ones_col, 1.0)
            # row-selector matrices: selN[k, j, m] = (k == j)
            sel2 = const.tile([BPC, BPC, 128], fp32, tag="sel2")
            nc.gpsimd.memset(sel2, 1.0)
            nc.gpsimd.affine_select(
                out=sel2,
                in_=sel2,
                compare_op=OP.is_equal,
                fill=0.0,
                base=0,
                pattern=[[-1, BPC], [0, 128]],
                channel_multiplier=1,
            )
            sel8 = const.tile([KD, KD, 128], fp32, tag="sel8")
            nc.gpsimd.memset(sel8, 1.0)
            nc.gpsimd.affine_select(
                out=sel8,
                in_=sel8,
                compare_op=OP.is_equal,
                fill=0.0,
                base=0,
                pattern=[[-1, KD], [0, 128]],
                channel_multiplier=1,
            )

            scale_sb = const.tile([1, 1], fp32, tag="scale")
            nc.sync.dma_start(out=scale_sb, in_=scale_d[None, :])
            # scale broadcast to all 128 partitions (via PE ones-matmul)
            scale128 = const.tile([128, 1], fp32, tag="scale128")
            psc = pb.tile([128, 512], fp32, tag="pb", name="psc")
            nc.tensor.matmul(psc[:, 0:1], ones1, scale_sb, start=True, stop=True)
            nc.scalar.copy(scale128, psc[:, 0:1])

            for _rep in range(repeat):
                # ---- DMA issue order: weights first (q2 critical path) ----
                # Wc then W, natural layout, 2 chunks per DMA (1MB each) so
                # the query/q2 pipelines track arrival
                WPD = 2  # 128-chunks per weight DMA
                NWT = KD // WPD
                wc_h = []
                for g in range(NWT):
                    w_ = wstream.tile(
                        [128, WPD, D], fp32, tag="wstream", name=f"wc{g}"
                    )
                    nc.sync.dma_start(
                        out=w_,
                        in_=wc_d[ts(g, WPD * 128), :].rearrange(
                            "(k p) e -> p k e", p=128
                        ),
                    )
                    wc_h.append(w_)
                w_h = []
                for g in range(NWT):
                    w_ = wstream.tile(
                        [128, WPD, D], fp32, tag="wstream", name=f"w{g}"
                    )
                    nc.sync.dma_start(
                        out=w_,
                        in_=w_d[ts(g, WPD * 128), :].rearrange(
                            "(k p) e -> p k e", p=128
                        ),
                    )
                    w_h.append(w_)
                wc_t = [wc_h[kc // WPD][:, kc % WPD, :] for kc in range(KD)]
                w_t = [w_h[kd // WPD][:, kd % WPD, :] for kd in range(KD)]

                # small inputs
                c_rows = const.tile([BPC, D], fp32, tag="c_rows")
                nc.sync.dma_start(out=c_rows, in_=c_d[:, :])
                bias_row = const.tile([1, D], fp32, tag="bias_row")
                nc.sync.dma_start(out=bias_row, in_=b_d[None, :])

                # mask (cast int32 -> f32 during DMA), layout matches eij cols
                mask_f = []
                for b in range(BPC):
                    mf = persist.tile([128, TS, QT], fp32, tag=f"mask{b}")
                    nc.gpsimd.dma_start(
                        out=mf,
                        in_=mask_d[b].rearrange("(t p q) -> p t q", p=128, q=QT),
                    )
                    mask_f.append(mf)

                # x tiles (the bulk: 16MB), issued after the weight DMAs
                x_tiles = [[None] * TS for _ in range(BPC)]
                for b in range(BPC if phase >= 2 else 0):
                    for t in range(TS):
                        xt = xp.tile([128, XF], f32r, tag="xt")
                        nc.sync.dma_start(
                            out=xt,
                            in_=x_d[b, ts(t, SBLK), :].rearrange(
                                "(p q) d -> p (q d)", p=128
                            ),
                        )
                        x_tiles[b][t] = xt

                # ---------- query = tanh(c @ Wc + bias) ----------
                # cT[p, kc, b] = c[b, 128kc+p] via PE transposes (c is tiny;
                # a strided gather DMA for this took ~15us to land)
                cT = const.tile([128, KD, BPC], fp32, tag="cT")
                for kc in range(KD):
                    ptc = pb.tile([128, 512], fp32, tag="pb", name="ptc")
                    nc.tensor.transpose(
                        ptc[:, 0:BPC], c_rows[:, ts(kc, 128)], identity[0:BPC, 0:BPC]
                    )
                    nc.scalar.copy(cT[:, kc, :], ptc[:, 0:BPC])

                # PE: lhsT = cT chunk [128,2] stationary, rhs = Wc chunk
                # [128,512] streaming; bias joins the chain as a rank-1 term.
                psum_q = pq.tile([BPC, D], fp32, tag="psum_q", name="psum_q")
                for kc in range(KD):
                    for h in range(2):
                        nc.tensor.matmul(
                            psum_q[:, ts(h, 512)],
                            cT[:, kc, :],
                            wc_t[kc][:, ts(h, 512)],
                            start=(kc == 0),
                            stop=False,
                        )
                for h in range(2):
                    nc.tensor.matmul(
                        psum_q[:, ts(h, 512)],
                        ones1[0:1, 0:BPC],
                        bias_row[0:1, ts(h, 512)],
                        start=False,
                        stop=True,
                    )
                q_pre = const.tile([BPC, D], fp32, tag="query_sb")
                nc.scalar.copy(q_pre, psum_q)

                # broadcast pre-activation rows to 128 partitions (selector
                # matmul), tanh applied on the wide [128,512] form
                qbc = []
                for b in range(BPC):
                    qb = persist.tile([128, D], fp32, tag=f"qbc{b}")
                    for h in range(2):
                        pbc = pb.tile([128, 512], fp32, tag="pb", name="pbc")
                        nc.tensor.matmul(
                            pbc,
                            sel2[:, b, :],
                            q_pre[:, ts(h, 512)],
                            start=True,
                            stop=True,
                        )
                        nc.scalar.activation(qb[:, ts(h, 512)], pbc, AF.Tanh)
                    qbc.append(qb)

                # ---------- q2 = W @ query on DVE + GpSimd ----------
                # q2col[b][p, kd] = sum_e W[128kd+p, e] * query[b, e]
                # DVE: fused STT; GP: TT(mult) + DVE reduce (offload)
                q2col = [
                    persist.tile([128, KD], fp32, tag=f"q2col{b}", name=f"q2col{b}")
                    for b in range(BPC)
                ]
                for kd in range(KD):
                    for b in range(BPC):
                        idx = kd * BPC + b
                        col = q2col[b][:, kd : kd + 1]
                        if idx % 8 < 5:
                            sc = pstt.tile([128, D], fp32, tag="stt_out", bufs=1)
                            nc.vector.scalar_tensor_tensor(
                                out=sc,
                                in0=w_t[kd],
                                scalar=1.0,
                                in1=qbc[b],
                                op0=OP.mult,
                                op1=OP.mult,
                                accum_out=col,
                            )
                        else:
                            prod = scratch.tile(
                                [128, D], fp32, tag="prod", bufs=2, name="prod"
                            )
                            nc.gpsimd.tensor_tensor(
                                prod, w_t[kd], qbc[b], op=OP.mult
                            )
                            nc.vector.tensor_reduce(
                                col, prod, axis=mybir.AxisListType.X, op=OP.add
                            )

                # transpose q2col -> [8,128]; selector-matmul broadcast with
                # `scale` folded into the copies (q2 is linear in scale)
                q2b = []
                for b in range(BPC):
                    pt = pb.tile([128, 512], fp32, tag="pb", name="ptq2")
                    nc.tensor.transpose(pt[0:KD, 0:128], q2col[b], identity)
                    q2t = const.tile([KD, 128], fp32, tag=f"q2t{b}")
                    nc.scalar.copy(q2t, pt[0:KD, 0:128])
                    qb = persist.tile([128, D], fp32, tag=f"q2b{b}")
                    for kd in range(KD):
                        pbc = pb.tile([128, 512], fp32, tag="pb", name="pbc2")
                        nc.tensor.matmul(
                            pbc[:, 0:128],
                            sel8[:, kd, :],
                            q2t,
                            start=True,
                            stop=True,
                        )
                        nc.scalar.mul(qb[:, ts(kd, 128)], pbc[:, 0:128], scale128)
                    q2b.append(qb)

                if phase == 1:
                    for b in range(BPC):
                        nc.sync.dma_start(out=out_d[b : b + 1, :], in_=q2b[b][0:1, :])

                # ---------- streaming: eij, softmax, pooling ----------
                out_sb = [
                    const.tile([1, D], fp32, tag=f"out_sb{b}", name=f"out_sb{b}")
                    for b in range(BPC if phase >= 5 else 0)
                ]

                for b in range(BPC if phase >= 3 else 0):
                    # eij[p, t, q] = <x[s], q2[b]>  for s = 512t + 4p + q
                    # split across DVE (10 ops) and GpSimd (6 ops) per batch
                    eij = persist.tile([128, TS, QT], fp32, tag=f"eij{b}")
                    n = 0
                    for t in range(TS):
                        for q in range(QT):
                            col = eij[:, t, q : q + 1]
                            if n % 8 < 5:
                                sc = pstt.tile(
                                    [128, D], fp32, tag="stt_out", bufs=1
                                )
                                nc.vector.scalar_tensor_tensor(
                                    out=sc,
                                    in0=f(x_tiles[b][t][:, ts(q, D)]),
                                    scalar=1.0,
                                    in1=q2b[b],
                                    op0=OP.mult,
                                    op1=OP.mult,
                                    accum_out=col,
                                )
                            else:
                                prod = scratch.tile(
                                    [128, D], fp32, tag="prod", bufs=2, name="prod"
                                )
                                nc.gpsimd.tensor_tensor(
                                    prod,
                                    f(x_tiles[b][t][:, ts(q, D)]),
                                    q2b[b],
                                    op=OP.mult,
                                )
                                nc.vector.tensor_reduce(
                                    col, prod, axis=mybir.AxisListType.X, op=OP.add
                                )
                            n += 1

                    if phase == 3:
                        nc.sync.dma_start(
                            out=out_d[b : b + 1, 0:16], in_=eij[0:1, :, :]
                        )
                        continue

                    # softmax (masked, unnormalized; normalization folded in)
                    m1 = scratch.tile([128, 1], fp32, tag="m1")
                    nc.vector.reduce_max(m1, eij, axis=mybir.AxisListType.XY)
                    pmax = pb.tile([1, 128], fp32, tag="pb", name="pmax")
                    nc.tensor.transpose(pmax, m1, identity)
                    negmx = scratch.tile([1, 1], fp32, tag="negmx")
                    nc.vector.reduce_max(
                        negmx, pmax, axis=mybir.AxisListType.X, negate=True
                    )
                    pbm = pb.tile([128, 512], fp32, tag="pb", name="pbm")
                    nc.tensor.matmul(pbm[:, 0:1], ones1, negmx, start=True, stop=True)
                    negm = scratch.tile([128, 1], fp32, tag="negm")
                    nc.scalar.copy(negm, pbm[:, 0:1])
                    a_b = persist.tile([128, TS, QT], fp32, tag=f"a{b}")
                    nc.scalar.activation(a_b, eij, AF.Exp, bias=negm, scale=1.0)
                    nc.vector.tensor_tensor(a_b, a_b, mask_f[b], op=OP.mult)
                    # f32r copy of a for the 1-cyc/row pooling matmul
                    # (engines cannot emit f32r; a casting DMA can)
                    a_r = persist.tile([128, TS, QT], f32r, tag=f"ar{b}", name="a_r")
                    nc.gpsimd.dma_start(out=a_r, in_=a_b)

                    # cross-partition sum via PE ones-matmul
                    s1 = scratch.tile([128, 1], fp32, tag="s1")
                    nc.vector.reduce_sum(s1, a_b, axis=mybir.AxisListType.XY)
                    ssum = pb.tile([1, 512], fp32, tag="pb", name="ssum")
                    nc.tensor.matmul(ssum[:, 0:1], s1, ones_col, start=True, stop=True)
                    den = scratch.tile([1, 1], fp32, tag="den")
                    nc.vector.tensor_scalar_add(den, ssum[:, 0:1], EPS)
                    rden = scratch.tile([1, 1], fp32, tag="rden")
                    nc.vector.reciprocal(rden, den)

                    if phase == 4:
                        nc.sync.dma_start(
                            out=out_d[b : b + 1, 0:16], in_=a_b[0:1, :, :]
                        )
                        continue

                    # out[b, d] = rden * sum_s a[s] x[s, d]   (f32r matmuls)
                    for h in range(2):
                        po = pp.tile([1, 512], fp32, tag="po", name="po")
                        n = 0
                        for t in range(TS):
                            for q in range(QT):
                                nc.tensor.matmul(
                                    po,
                                    a_r[:, t, q : q + 1],
                                    x_tiles[b][t][
                                        :, q * D + h * 512 : q * D + (h + 1) * 512
                                    ],
                                    start=(n == 0),
                                    stop=(n == TS * QT - 1),
                                )
                                n += 1
                        nc.vector.tensor_scalar_mul(
                            out_sb[b][:, ts(h, 512)], po, rden
                        )

                for b in range(BPC if phase >= 5 else 0):
                    nc.sync.dma_start(out=out_d[b : b + 1, :], in_=out_sb[b])

    nc.compile()
    return nc


def _get_nc():
    if "nc" not in _CACHE:
        _CACHE["nc"] = _build()
    return _CACHE["nc"]


def run(inputs, trace=False):
    from concourse.bass_utils import run_bass_kernel_spmd

    x = np.ascontiguousarray(inputs["x"], dtype=np.float32)
    mask = np.ascontiguousarray(inputs["mask"], dtype=np.int32)
    c = np.ascontiguousarray(inputs["c"], dtype=np.float32)
    W = np.ascontiguousarray(inputs["W"], dtype=np.float32)
    Wc = np.ascontiguousarray(inputs["Wc"], dtype=np.float32)
    b = np.ascontiguousarray(inputs["b"], dtype=np.float32)
    scale = np.ascontiguousarray(inputs["scale"], dtype=np.float32)

    in_maps = []
    for i in range(NCORES):
        sl = slice(i * BPC, (i + 1) * BPC)
        in_maps.append(
            {
                "x": x[sl],
                "mask": mask[sl],
                "c": c[sl],
                "W": W,
                "Wc": Wc,
                "b": b,
                "scale": scale,
            }
        )

    nc = _get_nc()
    res = run_bass_kernel_spmd(
        nc, in_maps, core_ids=list(range(NCORES)), trace=trace
    )
    out = np.concatenate([res.results[i]["out"] for i in range(NCORES)], axis=0)
    return out.astype(np.float32), res


def kernel(**inputs):
    out, _ = run(inputs, trace=False)
    return out


# revision 23
# speedup vs baseline: 1.0962x; 1.0962x over previous
"""Trainium2 Bass kernel for ContextAttentionMaskLuong.

Reference computation (per batch b):
    keys  = x @ W                       [B,S,D]
    query = tanh(c @ Wc + b)            [B,D]
    eij   = scale * <query, keys_s>     [B,S]
    a     = exp(eij - max) * mask; a /= (sum(a) + 1e-7)
    out   = sum_s a[s] * x[s,:]         [B,D]

Key rewrite: eij[b,s] = <x[b,s,:], q2[b]> with q2[b] = W @ (scale*query[b]),
which removes the [B,S,D]x[D,D] matmul entirely. The kernel is then one
streaming pass over x (memory-bound, ~25MB HBM read per core).

Sharding: data-parallel over batch: 16 batches / 8 cores = 2 per core.
W/Wc/b/scale replicated.

v2 design (engine balance; DMA is the roofline at ~80us):
- query = tanh(c @ Wc + bias): PE matmuls with tiny cT as the stationary
  operand and Wc streaming in natural layout (16 matmuls, f32r 1cyc/row).
- q2 = W @ (scale*query): fused mult+reduce on DVE/GpSimd with W in
  natural [d-part, e-free] layout (no PE transposes of W at all).
- partition broadcasts via selector-matrix matmuls (lhsT row-selector).
- eij via DVE+GpSimd scalar_tensor_tensor, split between both engines.
- softmax via reduce + PE transpose trick (tiny).
- pooling out = a^T @ x via PE matmuls in f32r (1 cyc/row).
- x tile DMAs issued after W/Wc so q2 is ready when x tiles land.

Per-core x layout (s-major): x tile t of batch b is SBUF [128, 4096]
where partition p, free q*1024+d  <->  x[b, 512*t + 4*p + q, d].
"""

import numpy as np

B, S, D = 16, 2048, 1024
NCORES = 8
BPC = B // NCORES  # batches per core
EPS = 1e-7

TS = 4  # x tiles per batch
QT = 4  # s-rows per partition per tile
XF = QT * D  # x tile free size (4096)
SBLK = S // TS  # s-block per tile (512)

_CACHE = {}


def _build():
    import os

    phase = int(os.environ.get("KPHASE", "5"))
    repeat = int(os.environ.get("KREPEAT", "1"))
    import concourse.bass as bass
    import concourse.mybir as mybir
    import concourse.tile as tile
    from concourse import bacc
    from concourse.masks import make_identity

    fp32 = mybir.dt.float32
    f32r = mybir.dt.float32r
    bf16 = mybir.dt.bfloat16
    i32 = mybir.dt.int32
    AF = mybir.ActivationFunctionType
    OP = mybir.AluOpType
    ts = bass.ts

    nc = bacc.Bacc(None)

    x_d = nc.dram_tensor("x", [BPC, S, D], f32r, kind="ExternalInput")
    mask_d = nc.dram_tensor("mask", [BPC, S], i32, kind="ExternalInput")
    c_d = nc.dram_tensor("c", [BPC, D], fp32, kind="ExternalInput")
    w_d = nc.dram_tensor("W", [D, D], fp32, kind="ExternalInput")
    wc_d = nc.dram_tensor("Wc", [D, D], fp32, kind="ExternalInput")
    b_d = nc.dram_tensor("b", [D], fp32, kind="ExternalInput")
    scale_d = nc.dram_tensor("scale", [1], fp32, kind="ExternalInput")
    out_d = nc.dram_tensor("out", [BPC, D], fp32, kind="ExternalOutput")

    KD = D // 128  # 8 chunks of 128 along d/e/c

    def r(ap):  # bitcast to f32r for 1-cycle/row PE matmuls
        return ap.bitcast(f32r)

    def f(ap):  # view f32r-declared data as plain f32 for DVE/ACT ops
        return ap.bitcast(fp32)

    with tile.TileContext(nc) as tc:
        with (
            tc.tile_pool(name="const", bufs=1) as const,
            tc.tile_pool(name="xp", bufs=2 * TS) as xp,
            tc.tile_pool(name="wstream", bufs=2) as wstream,
            tc.tile_pool(name="persist", bufs=1) as persist,
            tc.tile_pool(name="scratch", bufs=3) as scratch,
            tc.tile_pool(name="pq", bufs=1, space="PSUM") as pq,
            tc.tile_pool(name="pb", bufs=2, space="PSUM") as pb,
            tc.tile_pool(name="pp", bufs=2, space="PSUM") as pp,
            tc.tile_pool(name="pstt", bufs=1, space="PSUM") as pstt,
        ):
            # ---------- constants / small loads ----------
            identity = const.tile([128, 128], fp32, tag="identity")
            make_identity(nc, identity)
            ones1 = const.tile([1, 128], fp32, tag="ones1")
            nc.vector.memset(ones1, 1.0)
            ones_col = const.tile([128, 1], fp32, tag="ones_col")
            nc.vector.memset(ones_col, 1.0)
            # row-selector matrices: selN[k, j, m] = (k == j)
            sel2 = const.tile([BPC, BPC, 128], fp32, tag="sel2")
            nc.gpsimd.memset(sel2, 1.0)
            nc.gpsimd.affine_select(
                out=sel2,
                in_=sel2,
                compare_op=OP.is_equal,
                fill=0.0,
                base=0,
                pattern=[[-1, BPC], [0, 128]],
                channel_multiplier=1,
            )
            sel8 = const.tile([KD, KD, 128], fp32, tag="sel8")
            nc.gpsimd.memset(sel8, 1.0)
            nc.gpsimd.affine_select(
                out=sel8,
                in_=sel8,
                compare_op=OP.is_equal,
                fill=0.0,
                base=0,
                pattern=[[-1, KD], [0, 128]],
                channel_multiplier=1,
            )

            scale_sb = const.tile([1, 1], fp32, tag="scale")
            nc.sync.dma_start(out=scale_sb, in_=scale_d[None, :])
            # scale broadcast to all 128 partitions (via PE ones-matmul)
            scale128 = const.tile([128, 1], fp32, tag="scale128")
            psc = pb.tile([128, 512], fp32, tag="pb", name="psc")
            nc.tensor.matmul(psc[:, 0:1], ones1, scale_sb, start=True, stop=True)
            nc.scalar.copy(scale128, psc[:, 0:1])

            for _rep in range(repeat):
                # ---- DMA issue order: weights first (q2 critical path) ----
                # Wc then W, natural layout, 2 chunks per DMA (1MB each) so
                # the query/q2 pipelines track arrival
                WPD = 4  # 128-chunks per weight DMA
                NWT = KD // WPD
                wc_h = []
                for g in range(NWT):
                    w_ = wstream.tile(
                        [128, WPD, D], fp32, tag="wstream", name=f"wc{g}"
                    )
                    nc.sync.dma_start(
                        out=w_,
                        in_=wc_d[ts(g, WPD * 128), :].rearrange(
                            "(k p) e -> p k e", p=128
                        ),
                    )
                    wc_h.append(w_)
                w_h = []
                for g in range(NWT):
                    w_ = wstream.tile(
                        [128, WPD, D], fp32, tag="wstream", name=f"w{g}"
                    )
                    nc.sync.dma_start(
                        out=w_,
                        in_=w_d[ts(g, WPD * 128), :].rearrange(
                            "(k p) e -> p k e", p=128
                        ),
                    )
                    w_h.append(w_)
                wc_t = [wc_h[kc // WPD][:, kc % WPD, :] for kc in range(KD)]
                w_t = [w_h[kd // WPD][:, kd % WPD, :] for kd in range(KD)]

                # small inputs
                c_rows = const.tile([BPC, D], fp32, tag="c_rows")
                nc.sync.dma_start(out=c_rows, in_=c_d[:, :])
                bias_row = const.tile([1, D], fp32, tag="bias_row")
                nc.sync.dma_start(out=bias_row, in_=b_d[None, :])

                # mask (cast int32 -> f32 during DMA), layout matches eij cols
                mask_f = []
                for b in range(BPC):
                    mf = persist.tile([128, TS, QT], fp32, tag=f"mask{b}")
                    nc.gpsimd.dma_start(
                        out=mf,
                        in_=mask_d[b].rearrange("(t p q) -> p t q", p=128, q=QT),
                    )
                    mask_f.append(mf)

                # x tiles (the bulk: 16MB), issued after the weight DMAs
                x_tiles = [[None] * TS for _ in range(BPC)]
                for b in range(BPC if phase >= 2 else 0):
                    for t in range(TS):
                        xt = xp.tile([128, XF], f32r, tag="xt")
                        nc.sync.dma_start(
                            out=xt,
                            in_=x_d[b, ts(t, SBLK), :].rearrange(
                                "(p q) d -> p (q d)", p=128
                            ),
                        )
                        x_tiles[b][t] = xt

                # ---------- query = tanh(c @ Wc + bias) ----------
                # cT[p, kc, b] = c[b, 128kc+p] via PE transposes (c is tiny;
                # a strided gather DMA for this took ~15us to land)
                cT = const.tile([128, KD, BPC], fp32, tag="cT")
                for kc in range(KD):
                    ptc = pb.tile([128, 512], fp32, tag="pb", name="ptc")
                    nc.tensor.transpose(
                        ptc[:, 0:BPC], c_rows[:, ts(kc, 128)], identity[0:BPC, 0:BPC]
                    )
                    nc.scalar.copy(cT[:, kc, :], ptc[:, 0:BPC])

                # PE: lhsT = cT chunk [128,2] stationary, rhs = Wc chunk
                # [128,512] streaming; bias joins the chain as a rank-1 term.
                psum_q = pq.tile([BPC, D], fp32, tag="psum_q", name="psum_q")
                for kc in range(KD):
                    for h in range(2):
                        nc.tensor.matmul(
                            psum_q[:, ts(h, 512)],
                            cT[:, kc, :],
                            wc_t[kc][:, ts(h, 512)],
                            start=(kc == 0),
                            stop=False,
                        )
                for h in range(2):
                    nc.tensor.matmul(
                        psum_q[:, ts(h, 512)],
                        ones1[0:1, 0:BPC],
                        bias_row[0:1, ts(h, 512)],
                        start=False,
                        stop=True,
                    )
                q_pre = const.tile([BPC, D], fp32, tag="query_sb")
                nc.scalar.copy(q_pre, psum_q)

                # broadcast pre-activation rows to 128 partitions (selector
                # matmul), tanh applied on the wide [128,512] form
                qbc = []
                for b in range(BPC):
                    qb = persist.tile([128, D], fp32, tag=f"qbc{b}")
                    for h in range(2):
                        pbc = pb.tile([128, 512], fp32, tag="pb", name="pbc")
                        nc.tensor.matmul(
                            pbc,
                            sel2[:, b, :],
                            q_pre[:, ts(h, 512)],
                            start=True,
                            stop=True,
                        )
                        nc.scalar.activation(qb[:, ts(h, 512)], pbc, AF.Tanh)
                    qbc.append(qb)

                # ---------- q2 = W @ query on DVE + GpSimd ----------
                # q2col[b][p, kd] = sum_e W[128kd+p, e] * query[b, e]
                # DVE: fused STT; GP: TT(mult) + DVE reduce (offload)
                q2col = [
                    persist.tile([128, KD], fp32, tag=f"q2col{b}", name=f"q2col{b}")
                    for b in range(BPC)
                ]
                for kd in range(KD):
                    for b in range(BPC):
                        sc = pstt.tile([128, D], fp32, tag="stt_out", bufs=1)
                        nc.vector.scalar_tensor_tensor(
                            out=sc,
                            in0=w_t[kd],
                            scalar=1.0,
                            in1=qbc[b],
                            op0=OP.mult,
                            op1=OP.mult,
                            accum_out=q2col[b][:, kd : kd + 1],
                        )

                # transpose q2col -> [8,128]; selector-matmul broadcast with
                # `scale` folded into the copies (q2 is linear in scale)
                q2b = []
                for b in range(BPC):
                    pt = pb.tile([128, 512], fp32, tag="pb", name="ptq2")
                    nc.tensor.transpose(pt[0:KD, 0:128], q2col[b], identity)
                    q2t = const.tile([KD, 128], fp32, tag=f"q2t{b}")
                    nc.scalar.copy(q2t, pt[0:KD, 0:128])
                    qb = persist.tile([128, D], fp32, tag=f"q2b{b}")
                    for kd in range(KD):
                        pbc = pb.tile([128, 512], fp32, tag="pb", name="pbc2")
                        nc.tensor.matmul(
                            pbc[:, 0:128],
                            sel8[:, kd, :],
                            q2t,
                            start=True,
                            stop=True,
                        )
                        nc.scalar.mul(qb[:, ts(kd, 128)], pbc[:, 0:128], scale128)
                    q2b.append(qb)

                if phase == 1:
                    for b in range(BPC):
                        nc.sync.dma_start(out=out_d[b : b + 1, :], in_=q2b[b][0:1, :])

                # ---------- streaming: eij, softmax, pooling ----------
                out_sb = [
                    const.tile([1, D], fp32, tag=f"out_sb{b}", name=f"out_sb{b}")
                    for b in range(BPC if phase >= 5 else 0)
                ]

                for b in range(BPC if phase >= 3 else 0):
                    # eij[p, t, q] = <x[s], q2[b]>  for s = 512t + 4p + q
                    # split across DVE (10 ops) and GpSimd (6 ops) per batch
                    eij = persist.tile([128, TS, QT], fp32, tag=f"eij{b}")
                    for t in range(TS):
                        for q in range(QT):
                            sc = pstt.tile([128, D], fp32, tag="stt_out", bufs=1)
                            nc.vector.scalar_tensor_tensor(
                                out=sc,
                                in0=f(x_tiles[b][t][:, ts(q, D)]),
                                scalar=1.0,
                                in1=q2b[b],
                                op0=OP.mult,
                                op1=OP.mult,
                                accum_out=eij[:, t, q : q + 1],
                            )

                    if phase == 3:
                        nc.sync.dma_start(
                            out=out_d[b : b + 1, 0:16], in_=eij[0:1, :, :]
                        )
                        continue

                    # softmax (masked, unnormalized; normalization folded in)
                    m1 = scratch.tile([128, 1], fp32, tag="m1")
                    nc.vector.reduce_max(m1, eij, axis=mybir.AxisListType.XY)
                    pmax = pb.tile([1, 128], fp32, tag="pb", name="pmax")
                    nc.tensor.transpose(pmax, m1, identity)
                    negmx = scratch.tile([1, 1], fp32, tag="negmx")
                    nc.vector.reduce_max(
                        negmx, pmax, axis=mybir.AxisListType.X, negate=True
                    )
                    pbm = pb.tile([128, 512], fp32, tag="pb", name="pbm")
                    nc.tensor.matmul(pbm[:, 0:1], ones1, negmx, start=True, stop=True)
                    negm = scratch.tile([128, 1], fp32, tag="negm")
                    nc.scalar.copy(negm, pbm[:, 0:1])
                    a_b = persist.tile([128, TS, QT], fp32, tag=f"a{b}")
                    nc.scalar.activation(a_b, eij, AF.Exp, bias=negm, scale=1.0)
                    nc.vector.tensor_tensor(a_b, a_b, mask_f[b], op=OP.mult)
                    # f32r copy of a for the 1-cyc/row pooling matmul
                    # (engines cannot emit f32r; a casting DMA can)
                    a_r = persist.tile([128, TS, QT], f32r, tag=f"ar{b}", name="a_r")
                    nc.gpsimd.dma_start(out=a_r, in_=a_b)

                    # cross-partition sum via PE ones-matmul
                    s1 = scratch.tile([128, 1], fp32, tag="s1")
                    nc.vector.reduce_sum(s1, a_b, axis=mybir.AxisListType.XY)
                    ssum = pb.tile([1, 512], fp32, tag="pb", name="ssum")
                    nc.tensor.matmul(ssum[:, 0:1], s1, ones_col, start=True, stop=True)
                    den = scratch.tile([1, 1], fp32, tag="den")
                    nc.vector.tensor_scalar_add(den, ssum[:, 0:1], EPS)
                    rden = scratch.tile([1, 1], fp32, tag="rden")
                    nc.vector.reciprocal(rden, den)

                    if phase == 4:
                        nc.sync.dma_start(
                            out=out_d[b : b + 1, 0:16], in_=a_b[0:1, :, :]
                        )
                        continue

                    # out[b, d] = rden * sum_s a[s] x[s, d]   (f32r matmuls)
                    for h in range(2):
                        po = pp.tile([1, 512], fp32, tag="po", name="po")
                        n = 0
                        for t in range(TS):
                            for q in range(QT):
                                nc.tensor.matmul(
                                    po,
                                    a_r[:, t, q : q + 1],
                                    x_tiles[b][t][
                                        :, q * D + h * 512 : q * D + (h + 1) * 512
                                    ],
                                    start=(n == 0),
                                    stop=(n == TS * QT - 1),
                                )
                                n += 1
                        nc.vector.tensor_scalar_mul(
                            out_sb[b][:, ts(h, 512)], po, rden
                        )

                for b in range(BPC if phase >= 5 else 0):
                    nc.sync.dma_start(out=out_d[b : b + 1, :], in_=out_sb[b])

    nc.compile()
    return nc


def _get_nc():
    if "nc" not in _CACHE:
        _CACHE["nc"] = _build()
    return _CACHE["nc"]


def run(inputs, trace=False):
    from concourse.bass_utils import run_bass_kernel_spmd

    x = np.ascontiguousarray(inputs["x"], dtype=np.float32)
    mask = np.ascontiguousarray(inputs["mask"], dtype=np.int32)
    c = np.ascontiguousarray(inputs["c"], dtype=np.float32)
    W = np.ascontiguousarray(inputs["W"], dtype=np.float32)
    Wc = np.ascontiguousarray(inputs["Wc"], dtype=np.float32)
    b = np.ascontiguousarray(inputs["b"], dtype=np.float32)
    scale = np.ascontiguousarray(inputs["scale"], dtype=np.float32)

    in_maps = []
    for i in range(NCORES):
        sl = slice(i * BPC, (i + 1) * BPC)
        in_maps.append(
            {
                "x": x[sl],
                "mask": mask[sl],
                "c": c[sl],
                "W": W,
                "Wc": Wc,
                "b": b,
                "scale": scale,
            }
        )

    nc = _get_nc()
    res = run_bass_kernel_spmd(
        nc, in_maps, core_ids=list(range(NCORES)), trace=trace
    )
    out = np.concatenate([res.results[i]["out"] for i in range(NCORES)], axis=0)
    return out.astype(np.float32), res


def kernel(**inputs):
    out, _ = run(inputs, trace=False)
    return out


# revision 24
# speedup vs baseline: 1.1332x; 1.0338x over previous
"""Trainium2 Bass kernel for ContextAttentionMaskLuong.

Reference computation (per batch b):
    keys  = x @ W                       [B,S,D]
    query = tanh(c @ Wc + b)            [B,D]
    eij   = scale * <query, keys_s>     [B,S]
    a     = exp(eij - max) * mask; a /= (sum(a) + 1e-7)
    out   = sum_s a[s] * x[s,:]         [B,D]

Key rewrite: eij[b,s] = <x[b,s,:], q2[b]> with q2[b] = W @ (scale*query[b]),
which removes the [B,S,D]x[D,D] matmul entirely. The kernel is then one
streaming pass over x (memory-bound, ~25MB HBM read per core).

Sharding: data-parallel over batch: 16 batches / 8 cores = 2 per core.
W/Wc/b/scale replicated.

v2 design (engine balance; DMA is the roofline at ~80us):
- query = tanh(c @ Wc + bias): PE matmuls with tiny cT as the stationary
  operand and Wc streaming in natural layout (16 matmuls, f32r 1cyc/row).
- q2 = W @ (scale*query): fused mult+reduce on DVE/GpSimd with W in
  natural [d-part, e-free] layout (no PE transposes of W at all).
- partition broadcasts via selector-matrix matmuls (lhsT row-selector).
- eij via DVE+GpSimd scalar_tensor_tensor, split between both engines.
- softmax via reduce + PE transpose trick (tiny).
- pooling out = a^T @ x via PE matmuls in f32r (1 cyc/row).
- x tile DMAs issued after W/Wc so q2 is ready when x tiles land.

Per-core x layout (s-major): x tile t of batch b is SBUF [128, 4096]
where partition p, free q*1024+d  <->  x[b, 512*t + 4*p + q, d].
"""

import numpy as np

B, S, D = 16, 2048, 1024
NCORES = 8
BPC = B // NCORES  # batches per core
EPS = 1e-7

TS = 4  # x tiles per batch
QT = 4  # s-rows per partition per tile
XF = QT * D  # x tile free size (4096)
SBLK = S // TS  # s-block per tile (512)

_CACHE = {}


def _build():
    import os

    phase = int(os.environ.get("KPHASE", "5"))
    repeat = int(os.environ.get("KREPEAT", "1"))
    import concourse.bass as bass
    import concourse.mybir as mybir
    import concourse.tile as tile
    from concourse import bacc
    from concourse.masks import make_identity

    fp32 = mybir.dt.float32
    f32r = mybir.dt.float32r
    bf16 = mybir.dt.bfloat16
    i32 = mybir.dt.int32
    AF = mybir.ActivationFunctionType
    OP = mybir.AluOpType
    ts = bass.ts

    nc = bacc.Bacc(None)

    x_d = nc.dram_tensor("x", [BPC, S, D], f32r, kind="ExternalInput")
    mask_d = nc.dram_tensor("mask", [BPC, S], i32, kind="ExternalInput")
    c_d = nc.dram_tensor("c", [BPC, D], fp32, kind="ExternalInput")
    w_d = nc.dram_tensor("W", [D, D], fp32, kind="ExternalInput")
    wc_d = nc.dram_tensor("Wc", [D, D], fp32, kind="ExternalInput")
    b_d = nc.dram_tensor("b", [D], fp32, kind="ExternalInput")
    scale_d = nc.dram_tensor("scale", [1], fp32, kind="ExternalInput")
    out_d = nc.dram_tensor("out", [BPC, D], fp32, kind="ExternalOutput")

    KD = D // 128  # 8 chunks of 128 along d/e/c

    def r(ap):  # bitcast to f32r for 1-cycle/row PE matmuls
        return ap.bitcast(f32r)

    def f(ap):  # view f32r-declared data as plain f32 for DVE/ACT ops
        return ap.bitcast(fp32)

    with tile.TileContext(nc) as tc:
        with (
            tc.tile_pool(name="const", bufs=1) as const,
            tc.tile_pool(name="xp", bufs=2 * TS) as xp,
            tc.tile_pool(name="wstream", bufs=2) as wstream,
            tc.tile_pool(name="persist", bufs=1) as persist,
            tc.tile_pool(name="scratch", bufs=3) as scratch,
            tc.tile_pool(name="pq", bufs=1, space="PSUM") as pq,
            tc.tile_pool(name="pb", bufs=2, space="PSUM") as pb,
            tc.tile_pool(name="pp", bufs=2, space="PSUM") as pp,
            tc.tile_pool(name="pstt", bufs=1, space="PSUM") as pstt,
        ):
            # ---------- constants / small loads ----------
            identity = const.tile([128, 128], fp32, tag="identity")
            make_identity(nc, identity)
            ones1 = const.tile([1, 128], fp32, tag="ones1")
            nc.vector.memset(ones1, 1.0)
            ones_col = const.tile([128, 1], fp32, tag="ones_col")
            nc.vector.memset(ones_col, 1.0)
            # row-selector matrices: selN[k, j, m] = (k == j)
            sel2 = const.tile([BPC, BPC, 128], fp32, tag="sel2")
            nc.gpsimd.memset(sel2, 1.0)
            nc.gpsimd.affine_select(
                out=sel2,
                in_=sel2,
                compare_op=OP.is_equal,
                fill=0.0,
                base=0,
                pattern=[[-1, BPC], [0, 128]],
                channel_multiplier=1,
            )
            sel8 = const.tile([KD, KD, 128], fp32, tag="sel8")
            nc.gpsimd.memset(sel8, 1.0)
            nc.gpsimd.affine_select(
                out=sel8,
                in_=sel8,
                compare_op=OP.is_equal,
                fill=0.0,
                base=0,
                pattern=[[-1, KD], [0, 128]],
                channel_multiplier=1,
            )

            scale_sb = const.tile([1, 1], fp32, tag="scale")
            nc.sync.dma_start(out=scale_sb, in_=scale_d[None, :])
            # scale broadcast to all 128 partitions (via PE ones-matmul)
            scale128 = const.tile([128, 1], fp32, tag="scale128")
            psc = pb.tile([128, 512], fp32, tag="pb", name="psc")
            nc.tensor.matmul(psc[:, 0:1], ones1, scale_sb, start=True, stop=True)
            nc.scalar.copy(scale128, psc[:, 0:1])

            for _rep in range(repeat):
                # ---- DMA issue order ----
                # tiny inputs first (they gate the query chain), then
                # weights (q2 critical path), then the bulk x stream
                c_rows = const.tile([BPC, D], fp32, tag="c_rows")
                nc.sync.dma_start(out=c_rows, in_=c_d[:, :])
                bias_row = const.tile([1, D], fp32, tag="bias_row")
                nc.sync.dma_start(out=bias_row, in_=b_d[None, :])
                # weights
                # Wc then W, natural layout, 2 chunks per DMA (1MB each) so
                # the query/q2 pipelines track arrival
                WPD = 4  # 128-chunks per weight DMA
                NWT = KD // WPD
                wc_h = []
                for g in range(NWT):
                    w_ = wstream.tile(
                        [128, WPD, D], fp32, tag="wstream", name=f"wc{g}"
                    )
                    nc.sync.dma_start(
                        out=w_,
                        in_=wc_d[ts(g, WPD * 128), :].rearrange(
                            "(k p) e -> p k e", p=128
                        ),
                    )
                    wc_h.append(w_)
                w_h = []
                for g in range(NWT):
                    w_ = wstream.tile(
                        [128, WPD, D], fp32, tag="wstream", name=f"w{g}"
                    )
                    nc.sync.dma_start(
                        out=w_,
                        in_=w_d[ts(g, WPD * 128), :].rearrange(
                            "(k p) e -> p k e", p=128
                        ),
                    )
                    w_h.append(w_)
                wc_t = [wc_h[kc // WPD][:, kc % WPD, :] for kc in range(KD)]
                w_t = [w_h[kd // WPD][:, kd % WPD, :] for kd in range(KD)]

                # mask (cast int32 -> f32 during DMA), layout matches eij cols
                mask_f = []
                for b in range(BPC):
                    mf = persist.tile([128, TS, QT], fp32, tag=f"mask{b}")
                    nc.gpsimd.dma_start(
                        out=mf,
                        in_=mask_d[b].rearrange("(t p q) -> p t q", p=128, q=QT),
                    )
                    mask_f.append(mf)

                # x tiles (the bulk: 16MB), issued after the weight DMAs
                x_tiles = [[None] * TS for _ in range(BPC)]
                for b in range(BPC if phase >= 2 else 0):
                    for t in range(TS):
                        xt = xp.tile([128, XF], f32r, tag="xt")
                        nc.sync.dma_start(
                            out=xt,
                            in_=x_d[b, ts(t, SBLK), :].rearrange(
                                "(p q) d -> p (q d)", p=128
                            ),
                        )
                        x_tiles[b][t] = xt

                # ---------- query = tanh(c @ Wc + bias) ----------
                # cT[p, kc, b] = c[b, 128kc+p] via PE transposes (c is tiny;
                # a strided gather DMA for this took ~15us to land)
                cT = const.tile([128, KD, BPC], fp32, tag="cT")
                for kc in range(KD):
                    ptc = pb.tile([128, 512], fp32, tag="pb", name="ptc")
                    nc.tensor.transpose(
                        ptc[:, 0:BPC], c_rows[:, ts(kc, 128)], identity[0:BPC, 0:BPC]
                    )
                    nc.scalar.copy(cT[:, kc, :], ptc[:, 0:BPC])

                # PE: lhsT = cT chunk [128,2] stationary, rhs = Wc chunk
                # [128,512] streaming; bias joins the chain as a rank-1 term.
                psum_q = pq.tile([BPC, D], fp32, tag="psum_q", name="psum_q")
                for kc in range(KD):
                    for h in range(2):
                        nc.tensor.matmul(
                            psum_q[:, ts(h, 512)],
                            cT[:, kc, :],
                            wc_t[kc][:, ts(h, 512)],
                            start=(kc == 0),
                            stop=False,
                        )
                for h in range(2):
                    nc.tensor.matmul(
                        psum_q[:, ts(h, 512)],
                        ones1[0:1, 0:BPC],
                        bias_row[0:1, ts(h, 512)],
                        start=False,
                        stop=True,
                    )
                q_pre = const.tile([BPC, D], fp32, tag="query_sb")
                nc.scalar.copy(q_pre, psum_q)

                # broadcast pre-activation rows to 128 partitions (selector
                # matmul), tanh applied on the wide [128,512] form
                qbc = []
                for b in range(BPC):
                    qb = persist.tile([128, D], fp32, tag=f"qbc{b}")
                    for h in range(2):
                        pbc = pb.tile([128, 512], fp32, tag="pb", name="pbc")
                        nc.tensor.matmul(
                            pbc,
                            sel2[:, b, :],
                            q_pre[:, ts(h, 512)],
                            start=True,
                            stop=True,
                        )
                        nc.scalar.activation(qb[:, ts(h, 512)], pbc, AF.Tanh)
                    qbc.append(qb)

                # ---------- q2 = W @ query on DVE + GpSimd ----------
                # q2col[b][p, kd] = sum_e W[128kd+p, e] * query[b, e]
                # DVE: fused STT; GP: TT(mult) + DVE reduce (offload)
                q2col = [
                    persist.tile([128, KD], fp32, tag=f"q2col{b}", name=f"q2col{b}")
                    for b in range(BPC)
                ]
                for kd in range(KD):
                    for b in range(BPC):
                        sc = pstt.tile([128, D], fp32, tag="stt_out", bufs=1)
                        nc.vector.scalar_tensor_tensor(
                            out=sc,
                            in0=w_t[kd],
                            scalar=1.0,
                            in1=qbc[b],
                            op0=OP.mult,
                            op1=OP.mult,
                            accum_out=q2col[b][:, kd : kd + 1],
                        )

                # transpose q2col -> [8,128]; selector-matmul broadcast with
                # `scale` folded into the copies (q2 is linear in scale)
                q2b = []
                for b in range(BPC):
                    pt = pb.tile([128, 512], fp32, tag="pb", name="ptq2")
                    nc.tensor.transpose(pt[0:KD, 0:128], q2col[b], identity)
                    q2t = const.tile([KD, 128], fp32, tag=f"q2t{b}")
                    nc.scalar.copy(q2t, pt[0:KD, 0:128])
                    qb = persist.tile([128, D], fp32, tag=f"q2b{b}")
                    for kd in range(KD):
                        pbc = pb.tile([128, 512], fp32, tag="pb", name="pbc2")
                        nc.tensor.matmul(
                            pbc[:, 0:128],
                            sel8[:, kd, :],
                            q2t,
                            start=True,
                            stop=True,
                        )
                        nc.scalar.mul(qb[:, ts(kd, 128)], pbc[:, 0:128], scale128)
                    q2b.append(qb)

                if phase == 1:
                    for b in range(BPC):
                        nc.sync.dma_start(out=out_d[b : b + 1, :], in_=q2b[b][0:1, :])

                # ---------- streaming: eij, softmax, pooling ----------
                out_sb = [
                    const.tile([1, D], fp32, tag=f"out_sb{b}", name=f"out_sb{b}")
                    for b in range(BPC if phase >= 5 else 0)
                ]

                for b in range(BPC if phase >= 3 else 0):
                    # eij[p, t, q] = <x[s], q2[b]>  for s = 512t + 4p + q
                    # split across DVE (10 ops) and GpSimd (6 ops) per batch
                    eij = persist.tile([128, TS, QT], fp32, tag=f"eij{b}")
                    for t in range(TS):
                        for q in range(QT):
                            sc = pstt.tile([128, D], fp32, tag="stt_out", bufs=1)
                            nc.vector.scalar_tensor_tensor(
                                out=sc,
                                in0=f(x_tiles[b][t][:, ts(q, D)]),
                                scalar=1.0,
                                in1=q2b[b],
                                op0=OP.mult,
                                op1=OP.mult,
                                accum_out=eij[:, t, q : q + 1],
                            )

                    if phase == 3:
                        nc.sync.dma_start(
                            out=out_d[b : b + 1, 0:16], in_=eij[0:1, :, :]
                        )
                        continue

                    # softmax (masked, unnormalized; normalization folded in)
                    m1 = scratch.tile([128, 1], fp32, tag="m1")
                    nc.vector.reduce_max(m1, eij, axis=mybir.AxisListType.XY)
                    pmax = pb.tile([1, 128], fp32, tag="pb", name="pmax")
                    nc.tensor.transpose(pmax, m1, identity)
                    negmx = scratch.tile([1, 1], fp32, tag="negmx")
                    nc.vector.reduce_max(
                        negmx, pmax, axis=mybir.AxisListType.X, negate=True
                    )
                    pbm = pb.tile([128, 512], fp32, tag="pb", name="pbm")
                    nc.tensor.matmul(pbm[:, 0:1], ones1, negmx, start=True, stop=True)
                    negm = scratch.tile([128, 1], fp32, tag="negm")
                    nc.scalar.copy(negm, pbm[:, 0:1])
                    a_b = persist.tile([128, TS, QT], fp32, tag=f"a{b}")
                    nc.scalar.activation(a_b, eij, AF.Exp, bias=negm, scale=1.0)
                    nc.vector.tensor_tensor(a_b, a_b, mask_f[b], op=OP.mult)
                    # f32r copy of a for the 1-cyc/row pooling matmul
                    # (engines cannot emit f32r; a casting DMA can)
                    a_r = persist.tile([128, TS, QT], f32r, tag=f"ar{b}", name="a_r")
                    nc.gpsimd.dma_start(out=a_r, in_=a_b)

                    # cross-partition sum via PE ones-matmul
                    s1 = scratch.tile([128, 1], fp32, tag="s1")
                    nc.vector.reduce_sum(s1, a_b, axis=mybir.AxisListType.XY)
                    ssum = pb.tile([1, 512], fp32, tag="pb", name="ssum")
                    nc.tensor.matmul(ssum[:, 0:1], s1, ones_col, start=True, stop=True)
                    den = scratch.tile([1, 1], fp32, tag="den")
                    nc.vector.tensor_scalar_add(den, ssum[:, 0:1], EPS)
                    rden = scratch.tile([1, 1], fp32, tag="rden")
                    nc.vector.reciprocal(rden, den)

                    if phase == 4:
                        nc.sync.dma_start(
                            out=out_d[b : b + 1, 0:16], in_=a_b[0:1, :, :]
                        )
                        continue

                    # out[b, d] = rden * sum_s a[s] x[s, d]   (f32r matmuls)
                    for h in range(2):
                        po = pp.tile([1, 512], fp32, tag="po", name="po")
                        n = 0
                        for t in range(TS):
                            for q in range(QT):
                                nc.tensor.matmul(
                                    po,
                                    a_r[:, t, q : q + 1],
                                    x_tiles[b][t][
                                        :, q * D + h * 512 : q * D + (h + 1) * 512
                                    ],
                                    start=(n == 0),
                                    stop=(n == TS * QT - 1),
                                )
                                n += 1
                        nc.vector.tensor_scalar_mul(
                            out_sb[b][:, ts(h, 512)], po, rden
                        )

                for b in range(BPC if phase >= 5 else 0):
                    nc.sync.dma_start(out=out_d[b : b + 1, :], in_=out_sb[b])

    nc.compile()
    return nc


def _get_nc():
    if "nc" not in _CACHE:
        _CACHE["nc"] = _build()
    return _CACHE["nc"]


def run(inputs, trace=False):
    from concourse.bass_utils import run_bass_kernel_spmd

    x = np.ascontiguousarray(inputs["x"], dtype=np.float32)
    mask = np.ascontiguousarray(inputs["mask"], dtype=np.int32)
    c = np.ascontiguousarray(inputs["c"], dtype=np.float32)
    W = np.ascontiguousarray(inputs["W"], dtype=np.float32)
    Wc = np.ascontiguousarray(inputs["Wc"], dtype=np.float32)
    b = np.ascontiguousarray(inputs["b"], dtype=np.float32)
    scale = np.ascontiguousarray(inputs["scale"], dtype=np.float32)

    in_maps = []
    for i in range(NCORES):
        sl = slice(i * BPC, (i + 1) * BPC)
        in_maps.append(
            {
                "x": x[sl],
                "mask": mask[sl],
                "c": c[sl],
                "W": W,
                "Wc": Wc,
                "b": b,
                "scale": scale,
            }
        )

    nc = _get_nc()
    res = run_bass_kernel_spmd(
        nc, in_maps, core_ids=list(range(NCORES)), trace=trace
    )
    out = np.concatenate([res.results[i]["out"] for i in range(NCORES)], axis=0)
    return out.astype(np.float32), res


def kernel(**inputs):
    out, _ = run(inputs, trace=False)
    return out
